# revision 14
# baseline (speedup 1.0000x reference)
"""Cross-attention fusion kernel for Trainium2 (8 NeuronCores).

Reference computation (per sample b):
    q = Wq @ xs + bq            xs = x_s2[b] as [256, 4096]
    k = Wk @ xd + bk            xd = x_dem[b] as [64, 4096]
    v = Wv @ xd + bv
    attn = softmax_j(k^T q * c)             c = 256 ** -0.5
    out = v @ attn + x_s2[b]                out[ch, j] = sum_i v[ch, i] attn[i, j]

Device-side restructure (mathematically identical):
  - logits = k^T q * c = (M^T xd_aug)^T xs with M = [Wk^T; bk] @ (Wq * c) * 16
    precomputed on the host ([65, 256]); neither q nor k materializes. The
    extra x16 keeps kq out of fp8-subnormal range; the activation's free
    affine stage undoes it (scale=1/16).
  - bq adds a per-i constant to logits, which cancels in softmax_j -> dropped.
  - bk / bv folded in via a ones row appended to xd (contraction K=65).
  - softmax denominators folded into v columns (scale v[:, i] by 1/sum_j e)
    instead of scaling the whole e matrix.
  - exp is computed without a running-max shift: logits are O(1) here and the
    fp8 e-matrix is range-shifted by a fixed -ln(4) instead.
  - BOTH big matmuls run fp8e4 DoubleRow (K=256 per instruction):
    phase D contracts the 2 channel-chunks of kq/xs per instruction,
    phase E contracts i-block pairs of vts/e.

Schedule: the kernel is bound by the ACT engine's exp stream (~72us); the PE's
total work (~60us) is hidden under it by interleaving phase E with phase D.
PSUM: 6 banks double-buffer the logits (per-i-block chunks 1536/1536/1024 so
the exp stream never stalls), 2 banks rotate for A/B/E matmul outputs. Phase E
accumulates over i in 3 groups (6/6/4 i-blocks). Groups 0+1 fold into a bf16
accumulator that DMAs out as the exp stream ends; the last group evicts into a
separate output (summed on the host) so the tail is just 32 short DoubleRow
chains racing the eviction ring. Interleaved work is emitted one i-block
behind the logits matmuls so a stalled eviction never blocks the exp stream
(engine queues are in-order).

Sharding: 8 cores = 4 samples x 2 halves of the key-pixel axis i. Each core
emits partials out/out2 [256, 4096]; the host sums and adds the residual.
No collectives.
"""

import numpy as np
import ml_dtypes

import concourse.bass as bass
import concourse.mybir as mybir
import concourse.tile as tile
from concourse import bacc
from concourse.bass_utils import run_bass_kernel_spmd

P = 128
CH = 256          # out_ch == s2_ch
DEM = 64          # dem_ch
N = 4096          # pixels per sample (j axis)
NI = 2048         # key pixels per core (i axis, half of N)
KO = CH // P      # 2 partition chunks of the 256-channel axis
NIB = NI // P     # 16 i-blocks per core
NCORES = 8

F32 = mybir.dt.float32
BF16 = mybir.dt.bfloat16
FP8 = mybir.dt.float8e4
NP_BF16 = ml_dtypes.bfloat16
NP_FP8 = ml_dtypes.float8_e4m3   # IEEE e4m3 (max 240) == TRN FP8_EXP4
DR = mybir.MatmulPerfMode.DoubleRow

# fp8 scale plan for the out-matmul (phase E): e is stored as exp(z - ln4)
# (max ~166, inside e4m3 range) and vts as v * r * ALPHA_V (O(1) values).
# accum row sums are of exp(z - ln4), so r = 4/s; net output scale is
# 4 * ALPHA_V * (1/4) = ALPHA_V, undone at PSUM eviction.
ALPHA_V = 8192.0
E_BIAS = -1.3862943611198906  # -ln(4)
KQ_S = 16.0                   # kq stored as fp8 * 16; exp scale undoes it

# per-i-block exp chunks: 2 x 1536 + 1 x 1024 so the 6 PSUM banks of the
# logits pipeline double-buffer without stalling the ACT stream
DCH = ((0, 1536), (1536, 3072), (3072, 4096))
# phase-E i-accumulation groups, in DoubleRow pair units (2 i-blocks each).
# 4 groups of 2 pairs spread the E matmuls evenly across the exp stream
# (short chains also cycle the 2-bank psum ring faster).
GROUPS = ((0, 2), (2, 4), (4, 6), (6, 8))
# slot -> (group, unit_lo, unit_hi). Work scheduled at slot s is emitted
# after the D matmuls of i-block s+1 (one block of PE runway protects ACT).
E_SCHED = {}
for g in range(3):
    for k in range(4):
        E_SCHED[3 + 4 * g + k] = (g, 4 * k, 4 * k + 4)


def build_bass():
    nc = bacc.Bacc(None, target_bir_lowering=False)

    xs_d = nc.dram_tensor("xs", [CH, N], FP8, kind="ExternalInput")
    xda_d = nc.dram_tensor("xda", [DEM + 1, NI], BF16, kind="ExternalInput")
    wmv_d = nc.dram_tensor("wmv", [DEM + 1, 2 * CH], BF16, kind="ExternalInput")
    out_d = nc.dram_tensor("out", [CH, N], BF16, kind="ExternalOutput")
    out2_d = nc.dram_tensor("out2", [CH, N], BF16, kind="ExternalOutput")

    xs_v = xs_d.ap().rearrange("(ko p) j -> p ko j", p=P)
    out_v = out_d.ap().rearrange("(m p) j -> p m j", p=P)
    out2_v = out2_d.ap().rearrange("(m p) j -> p m j", p=P)

    mult = mybir.AluOpType.mult
    add = mybir.AluOpType.add

    with tile.TileContext(nc) as tc:
        with (
            tc.tile_pool(name="consts", bufs=1) as consts,
            tc.tile_pool(name="bigs", bufs=1) as bigs,
            tc.tile_pool(name="small", bufs=1) as small,
        ):
            xda_sb = consts.tile([DEM + 1, NI], BF16)
            wmv_sb = consts.tile([DEM + 1, 2 * CH], BF16)
            wm_sb = wmv_sb[:, 0:CH]
            wv_sb = wmv_sb[:, CH:2 * CH]

            xs_sb = bigs.tile([P, KO, N], FP8)
            kq_sb = bigs.tile([P, KO, NI], FP8)    # kq*16 [ci, i], fp8
            vt_sb = bigs.tile([P, NIB, CH], BF16)  # v^T[i, ch], i on partitions
            vts_sb = bigs.tile([P, NIB, CH], FP8)  # v^T * r * ALPHA_V
            e_sb = bigs.tile([P, NIB, N], FP8)     # exp(logits - ln4)[i, j]
            outf_sb = bigs.tile([P, KO, N], BF16)  # groups 0-2 accumulator
            outg_sb = bigs.tile([P, KO, N], BF16)  # group 3 partial

            sums_sb = small.tile([P, NIB, 3], F32)
            r_sb = small.tile([P, NIB], F32)
            ebias_sb = small.tile([P, 1], F32)
            warm_sb = small.tile([P, 512], BF16)
            dmy_sb = small.tile([P, 8], F32)
            dmy2_sb = small.tile([P, 8], F32)
            nc.vector.memset(ebias_sb, E_BIAS)
            nc.vector.memset(warm_sb, 0.0)
            nc.vector.memset(dmy_sb, 0.0)

            # Load the exp table set on ACT while the input DMAs fly.
            nc.scalar.activation(
                out=dmy2_sb, in_=dmy_sb,
                func=mybir.ActivationFunctionType.Exp, bias=ebias_sb,
            )

            # Few, large DMAs ordered by first use (issue is serial on the
            # sync sequencer, ~0.7us each; transfers fan out across queues).
            nc.sync.dma_start(out=wmv_sb, in_=wmv_d.ap())
            nc.sync.dma_start(out=xda_sb, in_=xda_d.ap())
            for jh in range(2):
                nc.sync.dma_start(
                    out=xs_sb[:, :, jh * 2048:(jh + 1) * 2048],
                    in_=xs_v[:, :, jh * 2048:(jh + 1) * 2048],
                )

            with (
                tc.tile_pool(name="dpsum", bufs=2, space="PSUM") as dpool,
                tc.tile_pool(name="epsum", bufs=2, space="PSUM") as epool,
            ):
                # Warm the PE's HAM clock gate with throwaway matmuls while
                # the input DMAs are in flight (~3.4us of PE activity flips
                # the clock from 1.2 to 2.4 GHz).
                wp = epool.tile([P, 512], F32, tag="e")
                for _ in range(6):
                    nc.tensor.matmul(
                        wp, lhsT=warm_sb[:, :P], rhs=warm_sb,
                        start=True, stop=True,
                    )

                # ---- Phase A: kq[ci, i] = sum_r M[r, ci] xd_aug[r, i] ----
                # kq column-chunk c feeds D i-blocks 4c..4c+3; c 0-1 up
                # front, c 2-3 deferred into the loop to unclog the early
                # DVE stream.
                def emit_a_chunk(c):
                    for m in range(KO):
                        ps = epool.tile([P, 512], F32, tag="e",
                                        name=f"a_{c}_{m}")
                        nc.tensor.matmul(
                            ps,
                            lhsT=wm_sb[:, m * P:(m + 1) * P],
                            rhs=xda_sb[:, c * 512:(c + 1) * 512],
                            start=True, stop=True,
                        )
                        nc.vector.tensor_copy(
                            out=kq_sb[:, m, c * 512:(c + 1) * 512], in_=ps
                        )

                for c in range(2):
                    emit_a_chunk(c)

                def emit_b_pair(u):
                    # v^T i-blocks u, u+1: [xd;1]^T @ [Wv^T; bv], two blocks
                    # per psum tile so one eviction serves both
                    ps = epool.tile([P, 512], F32, tag="e", name=f"b_{u}")
                    for t in range(2):
                        nc.tensor.matmul(
                            ps[:, t * CH:(t + 1) * CH],
                            lhsT=xda_sb[:, (u + t) * P:(u + t + 1) * P],
                            rhs=wv_sb,
                            start=True, stop=True,
                        )
                    nc.vector.tensor_copy(
                        out=vt_sb[:, u:u + 2, :],
                        in_=ps.rearrange("p (a b) -> p a b", a=2),
                    )

                def emit_softmax_scale(ib):
                    nc.vector.reduce_sum(
                        out=r_sb[:, ib:ib + 1],
                        in_=sums_sb[:, ib, :],
                        axis=mybir.AxisListType.X,
                    )
                    nc.vector.reciprocal(
                        out=r_sb[:, ib:ib + 1], in_=r_sb[:, ib:ib + 1]
                    )
                    nc.vector.tensor_scalar(
                        out=vts_sb[:, ib, :],
                        in0=vt_sb[:, ib, :],
                        scalar1=r_sb[:, ib:ib + 1],
                        scalar2=ALPHA_V,
                        op0=mult,
                        op1=mult,
                    )

                def emit_e_unit(g, unit, pool):
                    jn, m = unit >> 1, unit & 1
                    lo, hi = GROUPS[g]
                    if pool is dpool:
                        pew = dpool.tile([P, 1536], F32, tag="d",
                                         name=f"pe_{g}_{unit}")
                        pe = pew[:, 0:512]
                    else:
                        pe = epool.tile([P, 512], F32, tag="e",
                                        name=f"pe_{g}_{unit}")
                    for k, ibp in enumerate(range(lo, hi)):
                        nc.tensor.matmul(
                            pe,
                            lhsT=vts_sb[:, 2 * ibp:2 * ibp + 2,
                                        m * P:(m + 1) * P],
                            rhs=e_sb[:, 2 * ibp:2 * ibp + 2,
                                     jn * 512:(jn + 1) * 512],
                            start=(k == 0), stop=(ibp == hi - 1),
                            perf_mode=DR,
                        )
                    jr = slice(jn * 512, (jn + 1) * 512)
                    if g == 0:
                        nc.vector.tensor_scalar_mul(
                            out=outf_sb[:, m, jr], in0=pe,
                            scalar1=1.0 / ALPHA_V,
                        )
                    elif g < 3:
                        nc.vector.scalar_tensor_tensor(
                            out=outf_sb[:, m, jr], in0=pe,
                            scalar=1.0 / ALPHA_V, in1=outf_sb[:, m, jr],
                            op0=mult, op1=add,
                        )
                    elif m == 0:
                        nc.vector.tensor_scalar_mul(
                            out=outg_sb[:, m, jr], in0=pe,
                            scalar1=1.0 / ALPHA_V,
                        )
                    else:
                        # tail: ACT is idle once the exp stream ends, so it
                        # takes half the final evictions off the DVE
                        nc.scalar.mul(
                            out=outg_sb[:, m, jr], in_=pe, mul=1.0 / ALPHA_V
                        )

                # ---- Phase D: logits -> exp -> row sums. B, softmax scales
                # and phase-E chains trail the logits matmuls by one i-block
                # so the PE always has the next exp chunk queued first. ----
                for ib in range(NIB):
                    slot = ib - 1
                    units = E_SCHED.get(slot)
                    # the first slot of each group needs this slot's own vts
                    # (DVE-emitted after the exps), so its units must stay
                    # after emit_softmax_scale to keep the DVE queue acyclic
                    bunch = slot in (3, 7, 11)
                    kq_l = kq_sb[:, :, ib * P:(ib + 1) * P]
                    for ci, (j0, j1) in enumerate(DCH):
                        T = dpool.tile([P, 1536], F32, tag="d")
                        for jj in range((j1 - j0) // 512):
                            nc.tensor.matmul(
                                T[:, jj * 512:(jj + 1) * 512],
                                lhsT=kq_l,
                                rhs=xs_sb[:, :, j0 + jj * 512:j0 + (jj + 1) * 512],
                                start=True, stop=True,
                                perf_mode=DR,
                            )
                        nc.scalar.activation(
                            out=e_sb[:, ib, j0:j1],
                            in_=T[:, 0:j1 - j0],
                            func=mybir.ActivationFunctionType.Exp,
                            bias=ebias_sb,
                            scale=1.0 / KQ_S,
                            accum_out=sums_sb[:, ib, ci:ci + 1],
                        )
                        # spread interleaved work between the logits chunks
                        # so the PE queue never bunches behind a stalled unit
                        if ci == 0:
                            if 0 <= slot < 8:
                                emit_b_pair(2 * slot)
                            if slot == 2:
                                emit_a_chunk(2)
                            if slot == 6:
                                emit_a_chunk(3)
                            if units and not bunch:
                                g, lo, hi = units
                                for u in range(lo, min(lo + 2, hi)):
                                    emit_e_unit(g, u, epool)
                        elif ci == 1 and units and not bunch:
                            g, lo, hi = units
                            for u in range(lo + 2, hi):
                                emit_e_unit(g, u, epool)
                    if slot < 0:
                        continue
                    emit_softmax_scale(slot)
                    if units and bunch:
                        g, lo, hi = units
                        for u in range(lo, hi):
                            emit_e_unit(g, u, epool)

                # ---- tail: last softmax scale, outf DMA, last E group on a
                # 4-deep psum ring, out2 DMA per j-quarter ----
                emit_softmax_scale(NIB - 1)
                nc.sync.dma_start(out=out_v, in_=outf_sb)
                for unit in range(16):
                    emit_e_unit(3, unit, dpool if unit & 2 else epool)
                    if unit == 7:
                        nc.scalar.dma_start(
                            out=out2_v[:, :, 0:2048],
                            in_=outg_sb[:, :, 0:2048],
                        )
                nc.scalar.dma_start(
                    out=out2_v[:, :, 2048:4096],
                    in_=outg_sb[:, :, 2048:4096],
                )
    nc.finalize()
    return nc


_NC_CACHE = None


def _get_nc():
    global _NC_CACHE
    if _NC_CACHE is None:
        _NC_CACHE = build_bass()
    return _NC_CACHE


def make_in_maps(x_s2, x_dem, Wq, bq, Wk, bk, Wv, bv):
    scale = np.float32(CH ** -0.5)
    wk_aug = np.concatenate([Wk.T, bk[None, :]], axis=0)            # [65, 256]
    wm = (wk_aug @ (Wq * scale) * np.float32(KQ_S))                 # [65, 256]
    wv_aug = np.concatenate([Wv.T, bv[None, :]], axis=0)            # [65, 256]
    wmv = np.concatenate([wm, wv_aug], axis=1).astype(NP_BF16)      # [65, 512]
    ones = np.ones((1, NI), np.float32)
    in_maps = []
    for c in range(NCORES):
        s, h = divmod(c, 2)
        xs = np.ascontiguousarray(x_s2[s].reshape(CH, N)).astype(NP_FP8)
        xd = x_dem[s].reshape(DEM, N)[:, h * NI:(h + 1) * NI]
        xda = np.concatenate([xd, ones], axis=0).astype(NP_BF16)
        in_maps.append({"xs": xs, "xda": np.ascontiguousarray(xda),
                        "wmv": wmv})
    return in_maps


def run(inputs, trace=False, trace_cores=None):
    """Run the device kernel; returns (output, BassKernelResults)."""
    x_s2 = np.asarray(inputs["x_s2"], np.float32)
    x_dem = np.asarray(inputs["x_dem"], np.float32)
    args = {k: np.asarray(inputs[k], np.float32)
            for k in ("Wq", "bq", "Wk", "bk", "Wv", "bv")}
    in_maps = make_in_maps(x_s2, x_dem, args["Wq"], args["bq"],
                           args["Wk"], args["bk"], args["Wv"], args["bv"])
    nc = _get_nc()
    res = run_bass_kernel_spmd(nc, in_maps, core_ids=list(range(NCORES)),
                               trace=trace, trace_cores=trace_cores)
    B = x_s2.shape[0]
    out = np.empty_like(x_s2)
    for s in range(B):
        part = sum(res.results[c][o].astype(np.float32)
                   for c in (2 * s, 2 * s + 1) for o in ("out", "out2"))
        out[s] = part.reshape(CH, 64, 64) + x_s2[s]
    return out, res


def kernel(**inputs):
    out, _ = run(inputs, trace=False)
    return out
